# revision 10
# baseline (speedup 1.0000x reference)
"""Trainium2 Bass kernel for nn_DeltaAI_34703335752317 (gnn_message_passing).

Computation (see reference):
    x = relu(LN(V @ W1 + b1))   # [N, H], LN over H with eps=1e-5
    x = relu(LN(x @ W2 + b2))
    x = relu(LN(x @ W3 + b3))
    out[n] = dot(x[n], Wp[ilist[n], :, 0]) + bp[ilist[n]]
    out = where(sum|V[n]| == 0, marginals[ilist[n]], out) / temp

Strategy: pure data parallel over N across 8 cores.  Host pre-transposes V
(per-core packed [T, 128, VDIM] tiles so the contraction dim lands on SBUF
partitions with fully contiguous DMAs), folds the LN mean-centering into the
weights (z - mean(z) == V @ (W @ C) + b @ C with C = I - 1/H), and
pre-gathers the per-row output head Wp[ilist]/bp[ilist].  The device kernel
streams V^T tiles at HBM rate and runs matmuls + LN + head on chip.

All streamed data is fp16: halves HBM traffic vs fp32 and runs the PE at
1 cycle/row instead of fp32's 4 (fp32 matmuls issue as 2 half-speed passes).
PSUM accumulation and LN statistics stay fp32.  Verified numerically on the
host: fp16-chain max rel err ~1e-3 vs the 2e-2 gate (fp8 V was measured at
2.3e-2 — over the gate — hence fp16).
"""

import numpy as np

import concourse.bacc as bacc
import concourse.bass as bass
import concourse.tile as tile
from concourse import mybir
from concourse.bass import ts
from concourse.bass_utils import run_bass_kernel_spmd

NCORES = 8
N = 65536
VDIM = 2048
HDIM = 64
LN_EPS = 1e-5

NPC = N // NCORES          # rows per core = 8192
P = 128                    # partitions
TPC = NPC // P             # row-tiles per core = 64
GRP = 8                    # row-tiles per group (8*64 = 512 psum floats = 1 bank)
NG = TPC // GRP            # groups per core = 8
KC = VDIM // P             # contraction chunks = 16

F32 = mybir.dt.float32
F16 = mybir.dt.float16


def _build_nc(has_b, has_g, has_be, tpc=TPC, ng=NG):
    """Build + compile the per-core Bass program (same NEFF on all cores)."""
    TPC, NG = tpc, ng  # noqa: N806 — allow small-scale builds for simulation
    nc = bacc.Bacc(
        "TRN2", target_bir_lowering=False, debug=False, num_devices=NCORES
    )

    NRG = TPC // 4  # 512-row groups per core
    RG = 512        # rows per matmul moving operand (fp32 max free dim)
    vt = nc.dram_tensor("vt", [NRG, P, KC * RG], F16, kind="ExternalInput")
    w1 = nc.dram_tensor("w1", [VDIM, HDIM], F16, kind="ExternalInput")
    w2 = nc.dram_tensor("w2", [HDIM, HDIM], F16, kind="ExternalInput")
    w3 = nc.dram_tensor("w3", [HDIM, HDIM], F16, kind="ExternalInput")
    wg = nc.dram_tensor("wg", [NG, P, GRP, HDIM], F16, kind="ExternalInput")
    bg = nc.dram_tensor("bg", [NG, P, GRP], F32, kind="ExternalInput")
    ident = nc.dram_tensor("ident", [P, P], F16, kind="ExternalInput")
    b_in = g_in = be_in = None
    if has_b:
        b_in = nc.dram_tensor("bvec", [3, P, HDIM], F32, kind="ExternalInput")
    if has_g:
        g_in = nc.dram_tensor("gvec", [3, P, HDIM], F32, kind="ExternalInput")
    if has_be:
        be_in = nc.dram_tensor("bevec", [3, P, HDIM], F32, kind="ExternalInput")
    o = nc.dram_tensor("o", [NG, P, GRP], F32, kind="ExternalOutput")

    with tile.TileContext(nc) as tc:
        with (
            tc.tile_pool(name="consts", bufs=1) as consts,
            tc.tile_pool(name="vpool", bufs=8) as vpool,
            tc.tile_pool(name="xpool", bufs=6) as xpool,
            tc.tile_pool(name="upool", bufs=4) as upool,
            tc.tile_pool(name="sqpool", bufs=3) as sqpool,
            tc.tile_pool(name="xtpool", bufs=4) as xtpool,
            tc.tile_pool(name="wgpool", bufs=3) as wgpool,
            tc.tile_pool(name="stat", bufs=6) as stat,
            tc.tile_pool(name="respool", bufs=4) as respool,
            tc.tile_pool(name="psz", bufs=2, space="PSUM") as psz,
            tc.tile_pool(name="pzt", bufs=2, space="PSUM") as pzt,
            tc.tile_pool(name="ppt", bufs=2, space="PSUM") as ppt,
        ):
            # --- constants ---
            w1_sb = consts.tile([P, KC, HDIM], F16)
            nc.sync.dma_start(
                out=w1_sb[:], in_=w1[:].rearrange("(k p) h -> p k h", p=P)
            )
            w2_sb = consts.tile([HDIM, HDIM], F16)
            nc.sync.dma_start(out=w2_sb[:], in_=w2[:])
            w3_sb = consts.tile([HDIM, HDIM], F16)
            nc.sync.dma_start(out=w3_sb[:], in_=w3[:])
            id_sb = consts.tile([P, P], F16)
            nc.sync.dma_start(out=id_sb[:], in_=ident[:])
            eps_sb = consts.tile([P, 1], F32)
            nc.vector.memset(eps_sb[:], LN_EPS)
            b_sb = g_sb = be_sb = None
            if b_in is not None:
                b_sb = consts.tile([P, 3, HDIM], F32)
                nc.sync.dma_start(
                    out=b_sb[:], in_=b_in[:].rearrange("l p h -> p l h")
                )
            if g_in is not None:
                g_sb = consts.tile([P, 3, HDIM], F32)
                nc.sync.dma_start(
                    out=g_sb[:], in_=g_in[:].rearrange("l p h -> p l h")
                )
            if be_in is not None:
                be_sb = consts.tile([P, 3, HDIM], F32)
                nc.sync.dma_start(
                    out=be_sb[:], in_=be_in[:].rearrange("l p h -> p l h")
                )

            def ln_relu(pz, li):
                """LN (mean pre-folded into W) + relu: PSUM [P,GRP,H] -> SBUF."""
                w = pz
                if b_sb is not None:
                    wsb = upool.tile([P, GRP, HDIM], F32, tag="wsb")
                    nc.vector.tensor_add(
                        wsb[:],
                        pz[:],
                        b_sb[:, li, None, :].to_broadcast((P, GRP, HDIM)),
                    )
                    w = wsb
                sq = sqpool.tile([P, GRP, HDIM], F32)
                nc.scalar.square(sq[:], w[:])
                var = stat.tile([P, GRP], F32)
                nc.vector.reduce_sum(var[:], sq[:], axis=mybir.AxisListType.X)
                # std = sqrt(var/H + eps); inv = 1/std
                inv = stat.tile([P, GRP], F32)
                nc.scalar.activation(
                    inv[:],
                    var[:],
                    mybir.ActivationFunctionType.Sqrt,
                    bias=eps_sb[:],
                    scale=1.0 / HDIM,
                )
                nc.vector.reciprocal(inv[:], inv[:])
                u = upool.tile([P, GRP, HDIM], F32)
                nc.vector.tensor_mul(
                    u[:], w[:], inv[:, :, None].to_broadcast((P, GRP, HDIM))
                )
                if g_sb is not None:
                    nc.vector.tensor_mul(
                        u[:],
                        u[:],
                        g_sb[:, li, None, :].to_broadcast((P, GRP, HDIM)),
                    )
                if be_sb is not None:
                    nc.vector.tensor_add(
                        u[:],
                        u[:],
                        be_sb[:, li, None, :].to_broadcast((P, GRP, HDIM)),
                    )
                x = xpool.tile([P, GRP, HDIM], F16)
                nc.gpsimd.tensor_scalar_max(x[:], u[:], 0.0)
                return x

            for g in range(NG):
                vhs = []
                for half in range(2):
                    vh = vpool.tile([P, KC, RG], F16, tag="v")
                    nc.sync.dma_start(out=vh[:], in_=vt[2 * g + half])
                    vhs.append(vh)
                wg_sb = wgpool.tile([P, GRP, HDIM], F16)
                nc.sync.dma_start(out=wg_sb[:], in_=wg[g])
                bg_sb = respool.tile([P, GRP], F32, tag="bg")
                nc.sync.dma_start(out=bg_sb[:], in_=bg[g])

                # ---- layer 1: z^T = W1C^T @ V^T (W stationary, V^T moving) ----
                pzT = pzt.tile([HDIM, 2, RG], F32, tag="pzt")
                for half in range(2):
                    for k in range(KC):
                        nc.tensor.matmul(
                            pzT[:, half, :],
                            lhsT=w1_sb[:, k, :],
                            rhs=vhs[half][:, k, :],
                            start=(k == 0),
                            stop=(k == KC - 1),
                        )
                z1T = xtpool.tile([HDIM, 2, RG], F16, tag="xt")
                nc.scalar.copy(z1T[:], pzT[:])
                # transpose z^T back to rows-on-partitions [P, GRP, H]
                pz = psz.tile([P, GRP, HDIM], F16, tag="pz")
                for t in range(GRP):
                    nc.tensor.transpose(
                        pz[:, t, :],
                        z1T[:, t // 4, ts(t % 4, P)],
                        id_sb[:HDIM, :HDIM],
                    )
                x = ln_relu(pz, 0)

                # ---- layers 2,3: transpose x, then z = x @ W ----
                for li, w_sb in ((1, w2_sb), (2, w3_sb)):
                    pt = ppt.tile([HDIM, GRP, P], F16, tag="pt")
                    for t in range(GRP):
                        nc.tensor.transpose(pt[:, t, :], x[:, t, :], id_sb[:])
                    xt = xtpool.tile([HDIM, GRP, P], F16)
                    nc.scalar.copy(xt[:], pt[:])
                    pz2 = psz.tile([P, GRP, HDIM], F32, tag="pz")
                    for t in range(GRP):
                        nc.tensor.matmul(
                            pz2[:, t, :],
                            lhsT=xt[:, t, :],
                            rhs=w_sb[:],
                            start=True,
                            stop=True,
                        )
                    x = ln_relu(pz2, li)

                # ---- head: out = dot(x, wg) + bg ----
                scr = sqpool.tile([P, GRP, HDIM], F32, tag="scr")
                nc.gpsimd.tensor_mul(scr[:], x[:], wg_sb[:])
                dot = stat.tile([P, GRP], F32, tag="dot")
                nc.vector.reduce_sum(dot[:], scr[:], axis=mybir.AxisListType.X)
                res = respool.tile([P, GRP], F32, tag="res")
                nc.gpsimd.tensor_add(res[:], dot[:], bg_sb[:])
                nc.sync.dma_start(out=o[g], in_=res[:])

    nc.compile()
    return nc


_NC_CACHE = {}
LAST_RESULTS = None


def _get_nc(has_b, has_g, has_be):
    key = (has_b, has_g, has_be)
    if key not in _NC_CACHE:
        _NC_CACHE[key] = _build_nc(has_b, has_g, has_be)
    return _NC_CACHE[key]


def _center(w):
    # w @ (I - 1/H): subtract row-means, in float64 for exactness
    w64 = np.asarray(w, np.float64)
    return (w64 - w64.mean(axis=-1, keepdims=True)).astype(np.float32)


def kernel(
    V, ilist, temp, W1, b1, g1, be1, W2, b2, g2, be2, W3, b3, g3, be3,
    Wp, bp, marginals,
):
    V = np.asarray(V, np.float32)
    ilist_np = np.asarray(ilist)
    W1c = _center(np.asarray(W1)).astype(np.float16)
    W2c = _center(np.asarray(W2)).astype(np.float16)
    W3c = _center(np.asarray(W3)).astype(np.float16)
    bs = [np.asarray(b, np.float64) for b in (b1, b2, b3)]
    bs = np.stack([(b - b.mean()).astype(np.float32) for b in bs])  # [3, H]
    gs = np.stack([np.asarray(g, np.float32) for g in (g1, g2, g3)])
    bes = np.stack([np.asarray(b, np.float32) for b in (be1, be2, be3)])

    has_b = bool(np.any(bs))
    has_g = not bool(np.all(gs == 1.0))
    has_be = bool(np.any(bes))
    nc = _get_nc(has_b, has_g, has_be)

    # pre-gathered per-row output head
    Wg = np.ascontiguousarray(Wp[ilist_np, :, 0]).astype(np.float16)  # [N, H]
    bgv = np.ascontiguousarray(bp[ilist_np, 0, 0]).astype(np.float32)  # [N]

    shared = {
        "w1": W1c,
        "w2": W2c,
        "w3": W3c,
        "ident": np.eye(P, dtype=np.float16),
    }
    if has_b:
        shared["bvec"] = np.ascontiguousarray(
            np.broadcast_to(bs[:, None, :], (3, P, HDIM))
        )
    if has_g:
        shared["gvec"] = np.ascontiguousarray(
            np.broadcast_to(gs[:, None, :], (3, P, HDIM))
        )
    if has_be:
        shared["bevec"] = np.ascontiguousarray(
            np.broadcast_to(bes[:, None, :], (3, P, HDIM))
        )

    V16 = V.astype(np.float16)
    in_maps = []
    for c in range(NCORES):
        sl = slice(c * NPC, (c + 1) * NPC)
        # packed V^T row-groups: vt[rg, p, k*512 + r] = V[c*NPC + rg*512 + r, k*128 + p]
        vc = np.ascontiguousarray(
            V16[sl].reshape(TPC // 4, 512, KC, P).transpose(0, 3, 2, 1)
        ).reshape(TPC // 4, P, KC * 512)
        wgc = np.ascontiguousarray(
            Wg[sl].reshape(NG, GRP, P, HDIM).transpose(0, 2, 1, 3)
        )
        bgc = np.ascontiguousarray(
            bgv[sl].reshape(NG, GRP, P).transpose(0, 2, 1)
        )
        in_maps.append({"vt": vc, "wg": wgc, "bg": bgc, **shared})

    kres = run_bass_kernel_spmd(nc, in_maps, core_ids=list(range(NCORES)))
    global LAST_RESULTS
    LAST_RESULTS = kres
    out = np.empty(N, np.float32)
    for c in range(NCORES):
        oc = kres.results[c]["o"]  # [NG, P, GRP]
        out[c * NPC : (c + 1) * NPC] = oc.transpose(0, 2, 1).reshape(NPC)

    # epilogue on host: zero-row marginals + temperature
    zero_rows = np.abs(V).sum(axis=1) == 0.0
    if zero_rows.any():
        out = np.where(
            zero_rows, np.asarray(marginals, np.float32)[ilist_np], out
        )
    t = np.float32(np.asarray(temp))
    if t != 1.0:
        out = (out / t).astype(np.float32)
    return out


# revision 11
# speedup vs baseline: 2.0776x; 2.0776x over previous
"""Trainium2 Bass kernel for nn_DeltaAI_34703335752317 (gnn_message_passing).

Computation (see reference):
    x = relu(LN(V @ W1 + b1))   # [N, H], LN over H with eps=1e-5
    x = relu(LN(x @ W2 + b2))
    x = relu(LN(x @ W3 + b3))
    out[n] = dot(x[n], Wp[ilist[n], :, 0]) + bp[ilist[n]]
    out = where(sum|V[n]| == 0, marginals[ilist[n]], out) / temp

Strategy: pure data parallel over N across 8 cores.  Host pre-transposes V
(per-core packed [T, 128, VDIM] tiles so the contraction dim lands on SBUF
partitions with fully contiguous DMAs), folds the LN mean-centering into the
weights (z - mean(z) == V @ (W @ C) + b @ C with C = I - 1/H), and
pre-gathers the per-row output head Wp[ilist]/bp[ilist].  The device kernel
streams V^T tiles at HBM rate and runs matmuls + LN + head on chip.

All streamed data is fp16: halves HBM traffic vs fp32 and runs the PE at
1 cycle/row instead of fp32's 4 (fp32 matmuls issue as 2 half-speed passes).
PSUM accumulation and LN statistics stay fp32.  Verified numerically on the
host: fp16-chain max rel err ~1e-3 vs the 2e-2 gate (fp8 V was measured at
2.3e-2 — over the gate — hence fp16).
"""

import numpy as np

import concourse.bacc as bacc
import concourse.bass as bass
import concourse.tile as tile
from concourse import mybir
from concourse.bass import ts
from concourse.bass_utils import run_bass_kernel_spmd

NCORES = 8
N = 65536
VDIM = 2048
HDIM = 64
LN_EPS = 1e-5

NPC = N // NCORES          # rows per core = 8192
P = 128                    # partitions
TPC = NPC // P             # row-tiles per core = 64
GRP = 8                    # row-tiles per group (8*64 = 512 psum floats = 1 bank)
NG = TPC // GRP            # groups per core = 8
KC = VDIM // P             # contraction chunks = 16

F32 = mybir.dt.float32
F16 = mybir.dt.float16


def _build_nc(has_b, has_g, has_be, tpc=TPC, ng=NG):
    """Build + compile the per-core Bass program (same NEFF on all cores)."""
    TPC, NG = tpc, ng  # noqa: N806 — allow small-scale builds for simulation
    nc = bacc.Bacc(
        "TRN2", target_bir_lowering=False, debug=False, num_devices=NCORES
    )

    NRG = TPC // 4  # 512-row groups per core
    RG = 512        # rows per matmul moving operand (fp32 max free dim)
    vt = nc.dram_tensor("vt", [NRG, P, KC * RG], F16, kind="ExternalInput")
    w1 = nc.dram_tensor("w1", [VDIM, HDIM], F16, kind="ExternalInput")
    w2 = nc.dram_tensor("w2", [HDIM, HDIM], F16, kind="ExternalInput")
    w3 = nc.dram_tensor("w3", [HDIM, HDIM], F16, kind="ExternalInput")
    wg = nc.dram_tensor("wg", [NG, P, GRP, HDIM], F16, kind="ExternalInput")
    bg = nc.dram_tensor("bg", [NG, P, GRP], F32, kind="ExternalInput")
    ident = nc.dram_tensor("ident", [P, P], F16, kind="ExternalInput")
    b_in = g_in = be_in = None
    if has_b:
        b_in = nc.dram_tensor("bvec", [3, P, HDIM], F32, kind="ExternalInput")
    if has_g:
        g_in = nc.dram_tensor("gvec", [3, P, HDIM], F32, kind="ExternalInput")
    if has_be:
        be_in = nc.dram_tensor("bevec", [3, P, HDIM], F32, kind="ExternalInput")
    o = nc.dram_tensor("o", [NG, P, GRP], F32, kind="ExternalOutput")

    with tile.TileContext(nc) as tc:
        with (
            tc.tile_pool(name="consts", bufs=1) as consts,
            tc.tile_pool(name="vpool", bufs=8) as vpool,
            tc.tile_pool(name="xpool", bufs=6) as xpool,
            tc.tile_pool(name="upool", bufs=4) as upool,
            tc.tile_pool(name="sqpool", bufs=3) as sqpool,
            tc.tile_pool(name="xtpool", bufs=4) as xtpool,
            tc.tile_pool(name="wgpool", bufs=3) as wgpool,
            tc.tile_pool(name="stat", bufs=6) as stat,
            tc.tile_pool(name="respool", bufs=4) as respool,
            tc.tile_pool(name="psz", bufs=2, space="PSUM") as psz,
            tc.tile_pool(name="pzt", bufs=2, space="PSUM") as pzt,
            tc.tile_pool(name="ppt", bufs=2, space="PSUM") as ppt,
        ):
            # --- constants ---
            w1_sb = consts.tile([P, KC, HDIM], F16)
            nc.sync.dma_start(
                out=w1_sb[:], in_=w1[:].rearrange("(k p) h -> p k h", p=P)
            )
            w2_sb = consts.tile([HDIM, HDIM], F16)
            nc.sync.dma_start(out=w2_sb[:], in_=w2[:])
            w3_sb = consts.tile([HDIM, HDIM], F16)
            nc.sync.dma_start(out=w3_sb[:], in_=w3[:])
            id_sb = consts.tile([P, P], F16)
            nc.sync.dma_start(out=id_sb[:], in_=ident[:])
            eps_sb = consts.tile([P, 1], F32)
            nc.vector.memset(eps_sb[:], LN_EPS)
            b_sb = g_sb = be_sb = None
            if b_in is not None:
                b_sb = consts.tile([P, 3, HDIM], F32)
                nc.sync.dma_start(
                    out=b_sb[:], in_=b_in[:].rearrange("l p h -> p l h")
                )
            if g_in is not None:
                g_sb = consts.tile([P, 3, HDIM], F32)
                nc.sync.dma_start(
                    out=g_sb[:], in_=g_in[:].rearrange("l p h -> p l h")
                )
            if be_in is not None:
                be_sb = consts.tile([P, 3, HDIM], F32)
                nc.sync.dma_start(
                    out=be_sb[:], in_=be_in[:].rearrange("l p h -> p l h")
                )

            def ln_relu(pz, li):
                """LN (mean pre-folded into W) + relu: PSUM [P,GRP,H] -> SBUF."""
                w = pz
                if b_sb is not None:
                    wsb = upool.tile([P, GRP, HDIM], F32, tag="wsb")
                    nc.vector.tensor_add(
                        wsb[:],
                        pz[:],
                        b_sb[:, li, None, :].to_broadcast((P, GRP, HDIM)),
                    )
                    w = wsb
                sq = sqpool.tile([P, GRP, HDIM], F32)
                nc.scalar.square(sq[:], w[:])
                var = stat.tile([P, GRP], F32)
                nc.vector.reduce_sum(var[:], sq[:], axis=mybir.AxisListType.X)
                # std = sqrt(var/H + eps); inv = 1/std
                inv = stat.tile([P, GRP], F32)
                nc.scalar.activation(
                    inv[:],
                    var[:],
                    mybir.ActivationFunctionType.Sqrt,
                    bias=eps_sb[:],
                    scale=1.0 / HDIM,
                )
                nc.vector.reciprocal(inv[:], inv[:])
                u = upool.tile([P, GRP, HDIM], F32)
                nc.vector.tensor_mul(
                    u[:], w[:], inv[:, :, None].to_broadcast((P, GRP, HDIM))
                )
                if g_sb is not None:
                    nc.vector.tensor_mul(
                        u[:],
                        u[:],
                        g_sb[:, li, None, :].to_broadcast((P, GRP, HDIM)),
                    )
                if be_sb is not None:
                    nc.vector.tensor_add(
                        u[:],
                        u[:],
                        be_sb[:, li, None, :].to_broadcast((P, GRP, HDIM)),
                    )
                x = xpool.tile([P, GRP, HDIM], F16)
                nc.vector.tensor_scalar_max(x[:], u[:], 0.0)
                return x

            for g in range(NG):
                vhs = []
                for half in range(2):
                    vh = vpool.tile([P, KC, RG], F16, tag="v")
                    nc.sync.dma_start(out=vh[:], in_=vt[2 * g + half])
                    vhs.append(vh)
                wg_sb = wgpool.tile([P, GRP, HDIM], F16)
                nc.sync.dma_start(out=wg_sb[:], in_=wg[g])
                bg_sb = respool.tile([P, GRP], F32, tag="bg")
                nc.sync.dma_start(out=bg_sb[:], in_=bg[g])

                # ---- layer 1: z^T = W1C^T @ V^T (W stationary, V^T moving) ----
                pzT = pzt.tile([HDIM, 2, RG], F32, tag="pzt")
                for half in range(2):
                    for k in range(KC):
                        nc.tensor.matmul(
                            pzT[:, half, :],
                            lhsT=w1_sb[:, k, :],
                            rhs=vhs[half][:, k, :],
                            start=(k == 0),
                            stop=(k == KC - 1),
                        )
                z1T = xtpool.tile([HDIM, 2, RG], F16, tag="xt")
                nc.scalar.copy(z1T[:], pzT[:])
                # transpose z^T back to rows-on-partitions [P, GRP, H]
                pz = psz.tile([P, GRP, HDIM], F16, tag="pz")
                for t in range(GRP):
                    nc.tensor.transpose(
                        pz[:, t, :],
                        z1T[:, t // 4, ts(t % 4, P)],
                        id_sb[:HDIM, :HDIM],
                    )
                x = ln_relu(pz, 0)

                # ---- layers 2,3: transpose x, then z = x @ W ----
                for li, w_sb in ((1, w2_sb), (2, w3_sb)):
                    pt = ppt.tile([HDIM, GRP, P], F16, tag="pt")
                    for t in range(GRP):
                        nc.tensor.transpose(pt[:, t, :], x[:, t, :], id_sb[:])
                    xt = xtpool.tile([HDIM, GRP, P], F16)
                    nc.scalar.copy(xt[:], pt[:])
                    pz2 = psz.tile([P, GRP, HDIM], F32, tag="pz")
                    for t in range(GRP):
                        nc.tensor.matmul(
                            pz2[:, t, :],
                            lhsT=xt[:, t, :],
                            rhs=w_sb[:],
                            start=True,
                            stop=True,
                        )
                    x = ln_relu(pz2, li)

                # ---- head: out = dot(x, wg) + bg ----
                scr = sqpool.tile([P, GRP, HDIM], F32, tag="scr")
                nc.vector.tensor_mul(scr[:], x[:], wg_sb[:])
                dot = stat.tile([P, GRP], F32, tag="dot")
                nc.vector.reduce_sum(dot[:], scr[:], axis=mybir.AxisListType.X)
                res = respool.tile([P, GRP], F32, tag="res")
                nc.vector.tensor_add(res[:], dot[:], bg_sb[:])
                nc.sync.dma_start(out=o[g], in_=res[:])

    nc.compile()
    return nc


_NC_CACHE = {}
LAST_RESULTS = None


def _get_nc(has_b, has_g, has_be):
    key = (has_b, has_g, has_be)
    if key not in _NC_CACHE:
        _NC_CACHE[key] = _build_nc(has_b, has_g, has_be)
    return _NC_CACHE[key]


def _center(w):
    # w @ (I - 1/H): subtract row-means, in float64 for exactness
    w64 = np.asarray(w, np.float64)
    return (w64 - w64.mean(axis=-1, keepdims=True)).astype(np.float32)


def kernel(
    V, ilist, temp, W1, b1, g1, be1, W2, b2, g2, be2, W3, b3, g3, be3,
    Wp, bp, marginals,
):
    V = np.asarray(V, np.float32)
    ilist_np = np.asarray(ilist)
    W1c = _center(np.asarray(W1)).astype(np.float16)
    W2c = _center(np.asarray(W2)).astype(np.float16)
    W3c = _center(np.asarray(W3)).astype(np.float16)
    bs = [np.asarray(b, np.float64) for b in (b1, b2, b3)]
    bs = np.stack([(b - b.mean()).astype(np.float32) for b in bs])  # [3, H]
    gs = np.stack([np.asarray(g, np.float32) for g in (g1, g2, g3)])
    bes = np.stack([np.asarray(b, np.float32) for b in (be1, be2, be3)])

    has_b = bool(np.any(bs))
    has_g = not bool(np.all(gs == 1.0))
    has_be = bool(np.any(bes))
    nc = _get_nc(has_b, has_g, has_be)

    # pre-gathered per-row output head
    Wg = np.ascontiguousarray(Wp[ilist_np, :, 0]).astype(np.float16)  # [N, H]
    bgv = np.ascontiguousarray(bp[ilist_np, 0, 0]).astype(np.float32)  # [N]

    shared = {
        "w1": W1c,
        "w2": W2c,
        "w3": W3c,
        "ident": np.eye(P, dtype=np.float16),
    }
    if has_b:
        shared["bvec"] = np.ascontiguousarray(
            np.broadcast_to(bs[:, None, :], (3, P, HDIM))
        )
    if has_g:
        shared["gvec"] = np.ascontiguousarray(
            np.broadcast_to(gs[:, None, :], (3, P, HDIM))
        )
    if has_be:
        shared["bevec"] = np.ascontiguousarray(
            np.broadcast_to(bes[:, None, :], (3, P, HDIM))
        )

    V16 = V.astype(np.float16)
    in_maps = []
    for c in range(NCORES):
        sl = slice(c * NPC, (c + 1) * NPC)
        # packed V^T row-groups: vt[rg, p, k*512 + r] = V[c*NPC + rg*512 + r, k*128 + p]
        vc = np.ascontiguousarray(
            V16[sl].reshape(TPC // 4, 512, KC, P).transpose(0, 3, 2, 1)
        ).reshape(TPC // 4, P, KC * 512)
        wgc = np.ascontiguousarray(
            Wg[sl].reshape(NG, GRP, P, HDIM).transpose(0, 2, 1, 3)
        )
        bgc = np.ascontiguousarray(
            bgv[sl].reshape(NG, GRP, P).transpose(0, 2, 1)
        )
        in_maps.append({"vt": vc, "wg": wgc, "bg": bgc, **shared})

    kres = run_bass_kernel_spmd(nc, in_maps, core_ids=list(range(NCORES)))
    global LAST_RESULTS
    LAST_RESULTS = kres
    out = np.empty(N, np.float32)
    for c in range(NCORES):
        oc = kres.results[c]["o"]  # [NG, P, GRP]
        out[c * NPC : (c + 1) * NPC] = oc.transpose(0, 2, 1).reshape(NPC)

    # epilogue on host: zero-row marginals + temperature
    zero_rows = np.abs(V).sum(axis=1) == 0.0
    if zero_rows.any():
        out = np.where(
            zero_rows, np.asarray(marginals, np.float32)[ilist_np], out
        )
    t = np.float32(np.asarray(temp))
    if t != 1.0:
        out = (out / t).astype(np.float32)
    return out


# revision 13
# speedup vs baseline: 2.3353x; 1.1240x over previous
"""Trainium2 Bass kernel for nn_DeltaAI_34703335752317 (gnn_message_passing).

Computation (see reference):
    x = relu(LN(V @ W1 + b1))   # [N, H], LN over H with eps=1e-5
    x = relu(LN(x @ W2 + b2))
    x = relu(LN(x @ W3 + b3))
    out[n] = dot(x[n], Wp[ilist[n], :, 0]) + bp[ilist[n]]
    out = where(sum|V[n]| == 0, marginals[ilist[n]], out) / temp

Strategy: pure data parallel over N across 8 cores.  Host pre-transposes V
(per-core packed [T, 128, VDIM] tiles so the contraction dim lands on SBUF
partitions with fully contiguous DMAs), folds the LN mean-centering into the
weights (z - mean(z) == V @ (W @ C) + b @ C with C = I - 1/H), and
pre-gathers the per-row output head Wp[ilist]/bp[ilist].  The device kernel
streams V^T tiles at HBM rate and runs matmuls + LN + head on chip.

All streamed data is fp16: halves HBM traffic vs fp32 and runs the PE at
1 cycle/row instead of fp32's 4.  PSUM accumulation and LN statistics stay
fp32.  (fp8 V was measured at 2.3e-2 max rel err — over the 2e-2 gate.)

The per-group work is modulo-scheduled across 4 pipeline stages
(L1 | T1+LN1 | T2+MM2+LN2 | T3+MM3+LN3+head) so the in-order tensor-engine
queue never waits on a layer-norm chain: each stage's PE work for group g
lands one full iteration after the LN results it consumes.
"""

import numpy as np

import concourse.bacc as bacc
import concourse.bass as bass
import concourse.tile as tile
from concourse import mybir
from concourse.bass import ts
from concourse.bass_utils import run_bass_kernel_spmd

NCORES = 8
N = 65536
VDIM = 2048
HDIM = 64
LN_EPS = 1e-5

NPC = N // NCORES          # rows per core = 8192
P = 128                    # partitions
TPC = NPC // P             # row-tiles per core = 64
GRP = 8                    # row-tiles per group (8*64 = 512 psum floats = 1 bank)
NG = TPC // GRP            # groups per core = 8
KC = VDIM // P             # contraction chunks = 16
RG = 512                   # rows per matmul moving operand (psum bank)

F32 = mybir.dt.float32
F16 = mybir.dt.float16


def _build_nc(has_b, has_g, has_be, tpc=TPC, ng=NG):
    """Build + compile the per-core Bass program (same NEFF on all cores)."""
    TPC, NG = tpc, ng  # noqa: N806 — allow small-scale builds for simulation
    nc = bacc.Bacc(
        "TRN2", target_bir_lowering=False, debug=False, num_devices=NCORES
    )

    NRG = TPC // 4  # 512-row halves per core
    vt = nc.dram_tensor("vt", [NRG, P, KC * RG], F16, kind="ExternalInput")
    w1 = nc.dram_tensor("w1", [VDIM, HDIM], F16, kind="ExternalInput")
    w2 = nc.dram_tensor("w2", [HDIM, HDIM], F16, kind="ExternalInput")
    w3 = nc.dram_tensor("w3", [HDIM, HDIM], F16, kind="ExternalInput")
    wg = nc.dram_tensor("wg", [NG, P, GRP, HDIM], F16, kind="ExternalInput")
    bg = nc.dram_tensor("bg", [NG, P, GRP], F32, kind="ExternalInput")
    ident = nc.dram_tensor("ident", [P, P], F16, kind="ExternalInput")
    b_in = g_in = be_in = None
    if has_b:
        b_in = nc.dram_tensor("bvec", [3, P, HDIM], F32, kind="ExternalInput")
    if has_g:
        g_in = nc.dram_tensor("gvec", [3, P, HDIM], F32, kind="ExternalInput")
    if has_be:
        be_in = nc.dram_tensor("bevec", [3, P, HDIM], F32, kind="ExternalInput")
    o = nc.dram_tensor("o", [NG, P, GRP], F32, kind="ExternalOutput")

    with tile.TileContext(nc) as tc:
        with (
            tc.tile_pool(name="consts", bufs=1) as consts,
            tc.tile_pool(name="vpool", bufs=8) as vpool,
            tc.tile_pool(name="xpool", bufs=8) as xpool,
            tc.tile_pool(name="upool", bufs=4) as upool,
            tc.tile_pool(name="sqpool", bufs=4) as sqpool,
            tc.tile_pool(name="xtpool", bufs=6) as xtpool,
            tc.tile_pool(name="wgpool", bufs=3) as wgpool,
            tc.tile_pool(name="bgpool", bufs=3) as bgpool,
            tc.tile_pool(name="stat", bufs=8) as stat,
            tc.tile_pool(name="respool", bufs=3) as respool,
            tc.tile_pool(name="pzt", bufs=3, space="PSUM") as pzt,
            tc.tile_pool(name="ppt", bufs=3, space="PSUM") as ppt,
            tc.tile_pool(name="psz", bufs=2, space="PSUM") as psz,
        ):
            # --- constants ---
            w1_sb = consts.tile([P, KC, HDIM], F16)
            nc.sync.dma_start(
                out=w1_sb[:], in_=w1[:].rearrange("(k p) h -> p k h", p=P)
            )
            w2_sb = consts.tile([HDIM, HDIM], F16)
            nc.sync.dma_start(out=w2_sb[:], in_=w2[:])
            w3_sb = consts.tile([HDIM, HDIM], F16)
            nc.sync.dma_start(out=w3_sb[:], in_=w3[:])
            id_sb = consts.tile([P, P], F16)
            nc.sync.dma_start(out=id_sb[:], in_=ident[:])
            eps_sb = consts.tile([P, 1], F32)
            nc.vector.memset(eps_sb[:], LN_EPS)
            b_sb = g_sb = be_sb = None
            if b_in is not None:
                b_sb = consts.tile([P, 3, HDIM], F32)
                nc.sync.dma_start(
                    out=b_sb[:], in_=b_in[:].rearrange("l p h -> p l h")
                )
            if g_in is not None:
                g_sb = consts.tile([P, 3, HDIM], F32)
                nc.sync.dma_start(
                    out=g_sb[:], in_=g_in[:].rearrange("l p h -> p l h")
                )
            if be_in is not None:
                be_sb = consts.tile([P, 3, HDIM], F32)
                nc.sync.dma_start(
                    out=be_sb[:], in_=be_in[:].rearrange("l p h -> p l h")
                )

            def ln_relu(pz, li):
                """LN (mean pre-folded into W) + relu: PSUM [P,GRP,H] -> SBUF."""
                w = pz
                if b_sb is not None:
                    wsb = upool.tile([P, GRP, HDIM], F32, tag="wsb")
                    nc.vector.tensor_add(
                        wsb[:],
                        pz[:],
                        b_sb[:, li, None, :].to_broadcast((P, GRP, HDIM)),
                    )
                    w = wsb
                sq = sqpool.tile([P, GRP, HDIM], F32)
                nc.scalar.square(sq[:], w[:])
                var = stat.tile([P, GRP], F32)
                nc.vector.reduce_sum(var[:], sq[:], axis=mybir.AxisListType.X)
                # std = sqrt(var/H + eps); inv = 1/std
                inv = stat.tile([P, GRP], F32)
                nc.scalar.activation(
                    inv[:],
                    var[:],
                    mybir.ActivationFunctionType.Sqrt,
                    bias=eps_sb[:],
                    scale=1.0 / HDIM,
                )
                nc.vector.reciprocal(inv[:], inv[:])
                u = upool.tile([P, GRP, HDIM], F32)
                nc.vector.tensor_mul(
                    u[:], w[:], inv[:, :, None].to_broadcast((P, GRP, HDIM))
                )
                if g_sb is not None:
                    nc.vector.tensor_mul(
                        u[:],
                        u[:],
                        g_sb[:, li, None, :].to_broadcast((P, GRP, HDIM)),
                    )
                if be_sb is not None:
                    nc.vector.tensor_add(
                        u[:],
                        u[:],
                        be_sb[:, li, None, :].to_broadcast((P, GRP, HDIM)),
                    )
                x = xpool.tile([P, GRP, HDIM], F16)
                nc.vector.tensor_scalar_max(x[:], u[:], 0.0)
                return x

            def transpose_x(x):
                """x [P,GRP,H] -> xt [H,GRP,P] via PE transposes + copy."""
                pt = ppt.tile([HDIM, GRP, P], F16, tag="ppt")
                for t in range(GRP):
                    nc.tensor.transpose(pt[:, t, :], x[:, t, :], id_sb[:])
                xt = xtpool.tile([HDIM, GRP, P], F16, tag="xt")
                nc.scalar.copy(xt[:], pt[:])
                return xt

            def layer23(x, w_sb, li):
                """x @ W -> LN -> relu for layers 2/3."""
                xt = transpose_x(x)
                pz2 = psz.tile([P, GRP, HDIM], F32, tag="pz")
                for t in range(GRP):
                    nc.tensor.matmul(
                        pz2[:, t, :],
                        lhsT=xt[:, t, :],
                        rhs=w_sb[:],
                        start=True,
                        stop=True,
                    )
                return ln_relu(pz2, li)

            # ---- 4-stage modulo-scheduled pipeline over groups ----
            st = {}
            for i in range(NG + 3):
                if i < NG:
                    # stage 0: V loads + layer-1 matmuls (z^T halves in PSUM)
                    g = i
                    vhs, pzh = [], []
                    for half in range(2):
                        vh = vpool.tile([P, KC, RG], F16, tag="v")
                        nc.sync.dma_start(out=vh[:], in_=vt[2 * g + half])
                        vhs.append(vh)
                    for half in range(2):
                        ph = pzt.tile([HDIM, RG], F32, tag="pzt")
                        for k in range(KC):
                            nc.tensor.matmul(
                                ph[:],
                                lhsT=w1_sb[:, k, :],
                                rhs=vhs[half][:, k, :],
                                start=(k == 0),
                                stop=(k == KC - 1),
                            )
                        pzh.append(ph)
                    st[g] = {"pzh": pzh}

                g = i - 1
                if 0 <= g < NG:
                    # stage 1: copy z^T to SBUF fp16, un-transpose, LN1
                    s = st[g]
                    z1T = xtpool.tile([HDIM, 2, RG], F16, tag="xt")
                    nc.scalar.copy(z1T[:, 0, :], s["pzh"][0][:])
                    nc.scalar.copy(z1T[:, 1, :], s["pzh"][1][:])
                    pz = ppt.tile([P, GRP, HDIM], F16, tag="ppt")
                    for t in range(GRP):
                        nc.tensor.transpose(
                            pz[:, t, :],
                            z1T[:, t // 4, ts(t % 4, P)],
                            id_sb[:HDIM, :HDIM],
                        )
                    s["x"] = ln_relu(pz, 0)
                    # prefetch head operands (used at stage 3)
                    wg_sb = wgpool.tile([P, GRP, HDIM], F16)
                    nc.sync.dma_start(out=wg_sb[:], in_=wg[g])
                    bg_sb = bgpool.tile([P, GRP], F32)
                    nc.sync.dma_start(out=bg_sb[:], in_=bg[g])
                    s["wg"], s["bg"] = wg_sb, bg_sb

                g = i - 2
                if 0 <= g < NG:
                    # stage 2: layer 2
                    s = st[g]
                    s["x"] = layer23(s["x"], w2_sb, 1)

                g = i - 3
                if 0 <= g < NG:
                    # stage 3: layer 3 + head
                    s = st.pop(g)
                    x3 = layer23(s["x"], w3_sb, 2)
                    scr = sqpool.tile([P, GRP, HDIM], F32, tag="scr")
                    nc.vector.tensor_mul(scr[:], x3[:], s["wg"][:])
                    dot = stat.tile([P, GRP], F32, tag="dot")
                    nc.vector.reduce_sum(
                        dot[:], scr[:], axis=mybir.AxisListType.X
                    )
                    res = respool.tile([P, GRP], F32, tag="res")
                    nc.vector.tensor_add(res[:], dot[:], s["bg"][:])
                    nc.sync.dma_start(out=o[g], in_=res[:])

    nc.compile()
    return nc


_NC_CACHE = {}
LAST_RESULTS = None


def _get_nc(has_b, has_g, has_be):
    key = (has_b, has_g, has_be)
    if key not in _NC_CACHE:
        _NC_CACHE[key] = _build_nc(has_b, has_g, has_be)
    return _NC_CACHE[key]


def _center(w):
    # w @ (I - 1/H): subtract row-means, in float64 for exactness
    w64 = np.asarray(w, np.float64)
    return (w64 - w64.mean(axis=-1, keepdims=True)).astype(np.float32)


def kernel(
    V, ilist, temp, W1, b1, g1, be1, W2, b2, g2, be2, W3, b3, g3, be3,
    Wp, bp, marginals,
):
    V = np.asarray(V, np.float32)
    ilist_np = np.asarray(ilist)
    W1c = _center(np.asarray(W1)).astype(np.float16)
    W2c = _center(np.asarray(W2)).astype(np.float16)
    W3c = _center(np.asarray(W3)).astype(np.float16)
    bs = [np.asarray(b, np.float64) for b in (b1, b2, b3)]
    bs = np.stack([(b - b.mean()).astype(np.float32) for b in bs])  # [3, H]
    gs = np.stack([np.asarray(g, np.float32) for g in (g1, g2, g3)])
    bes = np.stack([np.asarray(b, np.float32) for b in (be1, be2, be3)])

    has_b = bool(np.any(bs))
    has_g = not bool(np.all(gs == 1.0))
    has_be = bool(np.any(bes))
    nc = _get_nc(has_b, has_g, has_be)

    # pre-gathered per-row output head
    Wg = np.ascontiguousarray(Wp[ilist_np, :, 0]).astype(np.float16)  # [N, H]
    bgv = np.ascontiguousarray(bp[ilist_np, 0, 0]).astype(np.float32)  # [N]

    shared = {
        "w1": W1c,
        "w2": W2c,
        "w3": W3c,
        "ident": np.eye(P, dtype=np.float16),
    }
    if has_b:
        shared["bvec"] = np.ascontiguousarray(
            np.broadcast_to(bs[:, None, :], (3, P, HDIM))
        )
    if has_g:
        shared["gvec"] = np.ascontiguousarray(
            np.broadcast_to(gs[:, None, :], (3, P, HDIM))
        )
    if has_be:
        shared["bevec"] = np.ascontiguousarray(
            np.broadcast_to(bes[:, None, :], (3, P, HDIM))
        )

    V16 = V.astype(np.float16)
    in_maps = []
    for c in range(NCORES):
        sl = slice(c * NPC, (c + 1) * NPC)
        # packed V^T row-halves: vt[rg, p, k*512 + r] = V[c*NPC + rg*512 + r, k*128 + p]
        vc = np.ascontiguousarray(
            V16[sl].reshape(TPC // 4, 512, KC, P).transpose(0, 3, 2, 1)
        ).reshape(TPC // 4, P, KC * 512)
        wgc = np.ascontiguousarray(
            Wg[sl].reshape(NG, GRP, P, HDIM).transpose(0, 2, 1, 3)
        )
        bgc = np.ascontiguousarray(
            bgv[sl].reshape(NG, GRP, P).transpose(0, 2, 1)
        )
        in_maps.append({"vt": vc, "wg": wgc, "bg": bgc, **shared})

    kres = run_bass_kernel_spmd(nc, in_maps, core_ids=list(range(NCORES)))
    global LAST_RESULTS
    LAST_RESULTS = kres
    out = np.empty(N, np.float32)
    for c in range(NCORES):
        oc = kres.results[c]["o"]  # [NG, P, GRP]
        out[c * NPC : (c + 1) * NPC] = oc.transpose(0, 2, 1).reshape(NPC)

    # epilogue on host: zero-row marginals + temperature
    zero_rows = np.abs(V).sum(axis=1) == 0.0
    if zero_rows.any():
        out = np.where(
            zero_rows, np.asarray(marginals, np.float32)[ilist_np], out
        )
    t = np.float32(np.asarray(temp))
    if t != 1.0:
        out = (out / t).astype(np.float32)
    return out


# revision 14
# speedup vs baseline: 2.5272x; 1.0822x over previous
"""Trainium2 Bass kernel for nn_DeltaAI_34703335752317 (gnn_message_passing).

Computation (see reference):
    x = relu(LN(V @ W1 + b1))   # [N, H], LN over H with eps=1e-5
    x = relu(LN(x @ W2 + b2))
    x = relu(LN(x @ W3 + b3))
    out[n] = dot(x[n], Wp[ilist[n], :, 0]) + bp[ilist[n]]
    out = where(sum|V[n]| == 0, marginals[ilist[n]], out) / temp

Strategy: pure data parallel over N across 8 cores.  Host pre-transposes V
(per-core packed [T, 128, VDIM] tiles so the contraction dim lands on SBUF
partitions with fully contiguous DMAs), folds the LN mean-centering into the
weights (z - mean(z) == V @ (W @ C) + b @ C with C = I - 1/H), and
pre-gathers the per-row output head Wp[ilist]/bp[ilist].  The device kernel
streams V^T tiles at HBM rate and runs matmuls + LN + head on chip.

All streamed data is fp16: halves HBM traffic vs fp32 and runs the PE at
1 cycle/row instead of fp32's 4.  PSUM accumulation and LN statistics stay
fp32.  (fp8 V was measured at 2.3e-2 max rel err — over the 2e-2 gate.)

The per-group work is modulo-scheduled across 4 pipeline stages
(L1 | T1+LN1 | T2+MM2+LN2 | T3+MM3+LN3+head) so the in-order tensor-engine
queue never waits on a layer-norm chain: each stage's PE work for group g
lands one full iteration after the LN results it consumes.
"""

import numpy as np

import concourse.bacc as bacc
import concourse.bass as bass
import concourse.tile as tile
from concourse import mybir
from concourse.bass import ts
from concourse.bass_utils import run_bass_kernel_spmd

NCORES = 8
N = 65536
VDIM = 2048
HDIM = 64
LN_EPS = 1e-5

NPC = N // NCORES          # rows per core = 8192
P = 128                    # partitions
TPC = NPC // P             # row-tiles per core = 64
GRP = 8                    # row-tiles per group (8*64 = 512 psum floats = 1 bank)
NG = TPC // GRP            # groups per core = 8
KC = VDIM // P             # contraction chunks = 16
RG = 512                   # rows per matmul moving operand (psum bank)

F32 = mybir.dt.float32
F16 = mybir.dt.float16


def _build_nc(has_b, has_g, has_be, tpc=TPC, ng=NG):
    """Build + compile the per-core Bass program (same NEFF on all cores)."""
    TPC, NG = tpc, ng  # noqa: N806 — allow small-scale builds for simulation
    nc = bacc.Bacc(
        "TRN2", target_bir_lowering=False, debug=False, num_devices=NCORES
    )

    NRG = TPC // 4  # 512-row halves per core
    vt = nc.dram_tensor("vt", [NRG, P, KC * RG], F16, kind="ExternalInput")
    w1 = nc.dram_tensor("w1", [VDIM, HDIM], F16, kind="ExternalInput")
    w2 = nc.dram_tensor("w2", [HDIM, HDIM], F16, kind="ExternalInput")
    w3 = nc.dram_tensor("w3", [HDIM, HDIM], F16, kind="ExternalInput")
    wg = nc.dram_tensor("wg", [NG, P, GRP, HDIM], F16, kind="ExternalInput")
    bg = nc.dram_tensor("bg", [NG, P, GRP], F32, kind="ExternalInput")
    ident = nc.dram_tensor("ident", [P, P], F16, kind="ExternalInput")
    b_in = g_in = be_in = None
    if has_b:
        b_in = nc.dram_tensor("bvec", [3, P, HDIM], F32, kind="ExternalInput")
    if has_g:
        g_in = nc.dram_tensor("gvec", [3, P, HDIM], F32, kind="ExternalInput")
    if has_be:
        be_in = nc.dram_tensor("bevec", [3, P, HDIM], F32, kind="ExternalInput")
    o = nc.dram_tensor("o", [NG, P, GRP], F32, kind="ExternalOutput")

    with tile.TileContext(nc) as tc:
        with (
            tc.tile_pool(name="consts", bufs=1) as consts,
            tc.tile_pool(name="vpool", bufs=8) as vpool,
            tc.tile_pool(name="xpool", bufs=8) as xpool,
            tc.tile_pool(name="upool", bufs=4) as upool,
            tc.tile_pool(name="sqpool", bufs=4) as sqpool,
            tc.tile_pool(name="xtpool", bufs=6) as xtpool,
            tc.tile_pool(name="wgpool", bufs=3) as wgpool,
            tc.tile_pool(name="bgpool", bufs=3) as bgpool,
            tc.tile_pool(name="stat", bufs=8) as stat,
            tc.tile_pool(name="respool", bufs=3) as respool,
            tc.tile_pool(name="pzt", bufs=3, space="PSUM") as pzt,
            tc.tile_pool(name="ppt", bufs=3, space="PSUM") as ppt,
            tc.tile_pool(name="psz", bufs=2, space="PSUM") as psz,
        ):
            # --- constants ---
            w1_sb = consts.tile([P, KC, HDIM], F16)
            nc.sync.dma_start(
                out=w1_sb[:], in_=w1[:].rearrange("(k p) h -> p k h", p=P)
            )
            w2_sb = consts.tile([HDIM, HDIM], F16)
            nc.sync.dma_start(out=w2_sb[:], in_=w2[:])
            w3_sb = consts.tile([HDIM, HDIM], F16)
            nc.sync.dma_start(out=w3_sb[:], in_=w3[:])
            id_sb = consts.tile([P, P], F16)
            nc.sync.dma_start(out=id_sb[:], in_=ident[:])
            eps_sb = consts.tile([P, 1], F32)
            nc.vector.memset(eps_sb[:], LN_EPS)
            b_sb = g_sb = be_sb = None
            if b_in is not None:
                b_sb = consts.tile([P, 3, HDIM], F32)
                nc.sync.dma_start(
                    out=b_sb[:], in_=b_in[:].rearrange("l p h -> p l h")
                )
            if g_in is not None:
                g_sb = consts.tile([P, 3, HDIM], F32)
                nc.sync.dma_start(
                    out=g_sb[:], in_=g_in[:].rearrange("l p h -> p l h")
                )
            if be_in is not None:
                be_sb = consts.tile([P, 3, HDIM], F32)
                nc.sync.dma_start(
                    out=be_sb[:], in_=be_in[:].rearrange("l p h -> p l h")
                )

            def ln_relu(pz, li):
                """LN (mean pre-folded into W) + relu: PSUM [P,GRP,H] -> SBUF."""
                w = pz
                if b_sb is not None:
                    wsb = upool.tile([P, GRP, HDIM], F32, tag="wsb")
                    nc.vector.tensor_add(
                        wsb[:],
                        pz[:],
                        b_sb[:, li, None, :].to_broadcast((P, GRP, HDIM)),
                    )
                    w = wsb
                sq = sqpool.tile([P, GRP, HDIM], F32)
                nc.scalar.square(sq[:], w[:])
                var = stat.tile([P, GRP], F32)
                nc.vector.reduce_sum(var[:], sq[:], axis=mybir.AxisListType.X)
                # std = sqrt(var/H + eps); inv = 1/std
                inv = stat.tile([P, GRP], F32)
                nc.scalar.activation(
                    inv[:],
                    var[:],
                    mybir.ActivationFunctionType.Sqrt,
                    bias=eps_sb[:],
                    scale=1.0 / HDIM,
                )
                nc.vector.reciprocal(inv[:], inv[:])
                u = upool.tile([P, GRP, HDIM], F32)
                nc.vector.tensor_mul(
                    u[:], w[:], inv[:, :, None].to_broadcast((P, GRP, HDIM))
                )
                if g_sb is not None:
                    nc.vector.tensor_mul(
                        u[:],
                        u[:],
                        g_sb[:, li, None, :].to_broadcast((P, GRP, HDIM)),
                    )
                if be_sb is not None:
                    nc.vector.tensor_add(
                        u[:],
                        u[:],
                        be_sb[:, li, None, :].to_broadcast((P, GRP, HDIM)),
                    )
                x = xpool.tile([P, GRP, HDIM], F16)
                nc.vector.tensor_scalar_max(x[:], u[:], 0.0)
                return x

            def transpose_x(x):
                """x [P,GRP,H] -> xt [H,GRP,P] via PE transposes + copy."""
                pt = ppt.tile([HDIM, GRP, P], F16, tag="ppt")
                for t in range(GRP):
                    nc.tensor.transpose(pt[:, t, :], x[:, t, :], id_sb[:])
                xt = xtpool.tile([HDIM, GRP, P], F16, tag="xt")
                nc.scalar.copy(xt[:], pt[:])
                return xt

            def layer23(x, w_sb, li):
                """x @ W -> LN -> relu for layers 2/3."""
                xt = transpose_x(x)
                pz2 = psz.tile([P, GRP, HDIM], F32, tag="pz")
                for t in range(GRP):
                    nc.tensor.matmul(
                        pz2[:, t, :],
                        lhsT=xt[:, t, :],
                        rhs=w_sb[:],
                        start=True,
                        stop=True,
                    )
                return ln_relu(pz2, li)

            # ---- 4-stage modulo-scheduled pipeline over groups ----
            # Stages are emitted oldest-first each iteration so that ops with
            # long waits (e.g. LN1's square, which needs this iteration's T1)
            # sit at the TAIL of each engine queue — in-order engines would
            # otherwise head-of-line-block the work the PE needs early.
            st = {}
            for i in range(NG + 3):
                if i < NG:
                    # V loads first so the SP queue issues them ASAP
                    g = i
                    vhs = []
                    for half in range(2):
                        vh = vpool.tile([P, KC, RG], F16, tag="v")
                        nc.sync.dma_start(out=vh[:], in_=vt[2 * g + half])
                        vhs.append(vh)
                    st[g] = {"vhs": vhs}

                g = i - 3
                if 0 <= g < NG:
                    # stage 3: layer 3 + head
                    s = st.pop(g)
                    x3 = layer23(s["x"], w3_sb, 2)
                    scr = sqpool.tile([P, GRP, HDIM], F32, tag="scr")
                    nc.vector.tensor_mul(scr[:], x3[:], s["wg"][:])
                    dot = stat.tile([P, GRP], F32, tag="dot")
                    nc.vector.reduce_sum(
                        dot[:], scr[:], axis=mybir.AxisListType.X
                    )
                    res = respool.tile([P, GRP], F32, tag="res")
                    nc.vector.tensor_add(res[:], dot[:], s["bg"][:])
                    nc.sync.dma_start(out=o[g], in_=res[:])

                g = i - 2
                if 0 <= g < NG:
                    # stage 2: layer 2
                    s = st[g]
                    s["x"] = layer23(s["x"], w2_sb, 1)

                g = i - 1
                if 0 <= g < NG:
                    # stage 1: copy z^T to SBUF fp16, un-transpose, LN1
                    s = st[g]
                    z1T = xtpool.tile([HDIM, 2, RG], F16, tag="xt")
                    nc.scalar.copy(z1T[:, 0, :], s["pzh"][0][:])
                    nc.scalar.copy(z1T[:, 1, :], s["pzh"][1][:])
                    pz = ppt.tile([P, GRP, HDIM], F16, tag="ppt")
                    for t in range(GRP):
                        nc.tensor.transpose(
                            pz[:, t, :],
                            z1T[:, t // 4, ts(t % 4, P)],
                            id_sb[:HDIM, :HDIM],
                        )
                    s["x"] = ln_relu(pz, 0)
                    # prefetch head operands (used at stage 3)
                    wg_sb = wgpool.tile([P, GRP, HDIM], F16)
                    nc.sync.dma_start(out=wg_sb[:], in_=wg[g])
                    bg_sb = bgpool.tile([P, GRP], F32)
                    nc.sync.dma_start(out=bg_sb[:], in_=bg[g])
                    s["wg"], s["bg"] = wg_sb, bg_sb

                if i < NG:
                    # stage 0 (PE last): layer-1 matmuls (z^T halves in PSUM)
                    g = i
                    s = st[g]
                    pzh = []
                    for half in range(2):
                        ph = pzt.tile([HDIM, RG], F32, tag="pzt")
                        for k in range(KC):
                            nc.tensor.matmul(
                                ph[:],
                                lhsT=w1_sb[:, k, :],
                                rhs=s["vhs"][half][:, k, :],
                                start=(k == 0),
                                stop=(k == KC - 1),
                            )
                        pzh.append(ph)
                    s["pzh"] = pzh

    nc.compile()
    return nc


_NC_CACHE = {}
LAST_RESULTS = None


def _get_nc(has_b, has_g, has_be):
    key = (has_b, has_g, has_be)
    if key not in _NC_CACHE:
        _NC_CACHE[key] = _build_nc(has_b, has_g, has_be)
    return _NC_CACHE[key]


def _center(w):
    # w @ (I - 1/H): subtract row-means, in float64 for exactness
    w64 = np.asarray(w, np.float64)
    return (w64 - w64.mean(axis=-1, keepdims=True)).astype(np.float32)


def kernel(
    V, ilist, temp, W1, b1, g1, be1, W2, b2, g2, be2, W3, b3, g3, be3,
    Wp, bp, marginals,
):
    V = np.asarray(V, np.float32)
    ilist_np = np.asarray(ilist)
    W1c = _center(np.asarray(W1)).astype(np.float16)
    W2c = _center(np.asarray(W2)).astype(np.float16)
    W3c = _center(np.asarray(W3)).astype(np.float16)
    bs = [np.asarray(b, np.float64) for b in (b1, b2, b3)]
    bs = np.stack([(b - b.mean()).astype(np.float32) for b in bs])  # [3, H]
    gs = np.stack([np.asarray(g, np.float32) for g in (g1, g2, g3)])
    bes = np.stack([np.asarray(b, np.float32) for b in (be1, be2, be3)])

    has_b = bool(np.any(bs))
    has_g = not bool(np.all(gs == 1.0))
    has_be = bool(np.any(bes))
    nc = _get_nc(has_b, has_g, has_be)

    # pre-gathered per-row output head
    Wg = np.ascontiguousarray(Wp[ilist_np, :, 0]).astype(np.float16)  # [N, H]
    bgv = np.ascontiguousarray(bp[ilist_np, 0, 0]).astype(np.float32)  # [N]

    shared = {
        "w1": W1c,
        "w2": W2c,
        "w3": W3c,
        "ident": np.eye(P, dtype=np.float16),
    }
    if has_b:
        shared["bvec"] = np.ascontiguousarray(
            np.broadcast_to(bs[:, None, :], (3, P, HDIM))
        )
    if has_g:
        shared["gvec"] = np.ascontiguousarray(
            np.broadcast_to(gs[:, None, :], (3, P, HDIM))
        )
    if has_be:
        shared["bevec"] = np.ascontiguousarray(
            np.broadcast_to(bes[:, None, :], (3, P, HDIM))
        )

    V16 = V.astype(np.float16)
    in_maps = []
    for c in range(NCORES):
        sl = slice(c * NPC, (c + 1) * NPC)
        # packed V^T row-halves: vt[rg, p, k*512 + r] = V[c*NPC + rg*512 + r, k*128 + p]
        vc = np.ascontiguousarray(
            V16[sl].reshape(TPC // 4, 512, KC, P).transpose(0, 3, 2, 1)
        ).reshape(TPC // 4, P, KC * 512)
        wgc = np.ascontiguousarray(
            Wg[sl].reshape(NG, GRP, P, HDIM).transpose(0, 2, 1, 3)
        )
        bgc = np.ascontiguousarray(
            bgv[sl].reshape(NG, GRP, P).transpose(0, 2, 1)
        )
        in_maps.append({"vt": vc, "wg": wgc, "bg": bgc, **shared})

    kres = run_bass_kernel_spmd(nc, in_maps, core_ids=list(range(NCORES)))
    global LAST_RESULTS
    LAST_RESULTS = kres
    out = np.empty(N, np.float32)
    for c in range(NCORES):
        oc = kres.results[c]["o"]  # [NG, P, GRP]
        out[c * NPC : (c + 1) * NPC] = oc.transpose(0, 2, 1).reshape(NPC)

    # epilogue on host: zero-row marginals + temperature
    zero_rows = np.abs(V).sum(axis=1) == 0.0
    if zero_rows.any():
        out = np.where(
            zero_rows, np.asarray(marginals, np.float32)[ilist_np], out
        )
    t = np.float32(np.asarray(temp))
    if t != 1.0:
        out = (out / t).astype(np.float32)
    return out


# revision 15
# speedup vs baseline: 2.5583x; 1.0123x over previous
"""Trainium2 Bass kernel for nn_DeltaAI_34703335752317 (gnn_message_passing).

Computation (see reference):
    x = relu(LN(V @ W1 + b1))   # [N, H], LN over H with eps=1e-5
    x = relu(LN(x @ W2 + b2))
    x = relu(LN(x @ W3 + b3))
    out[n] = dot(x[n], Wp[ilist[n], :, 0]) + bp[ilist[n]]
    out = where(sum|V[n]| == 0, marginals[ilist[n]], out) / temp

Strategy: pure data parallel over N across 8 cores.  Host pre-transposes V
(per-core packed [T, 128, VDIM] tiles so the contraction dim lands on SBUF
partitions with fully contiguous DMAs), folds the LN mean-centering into the
weights (z - mean(z) == V @ (W @ C) + b @ C with C = I - 1/H), and
pre-gathers the per-row output head Wp[ilist]/bp[ilist].  The device kernel
streams V^T tiles at HBM rate and runs matmuls + LN + head on chip.

All streamed data is fp16: halves HBM traffic vs fp32 and runs the PE at
1 cycle/row instead of fp32's 4.  PSUM accumulation and LN statistics stay
fp32.  (fp8 V was measured at 2.3e-2 max rel err — over the 2e-2 gate.)

The per-group work is modulo-scheduled across 4 pipeline stages
(L1 | T1+LN1 | T2+MM2+LN2 | T3+MM3+LN3+head) so the in-order tensor-engine
queue never waits on a layer-norm chain: each stage's PE work for group g
lands one full iteration after the LN results it consumes.
"""

import numpy as np

import concourse.bacc as bacc
import concourse.bass as bass
import concourse.tile as tile
from concourse import mybir
from concourse.bass import ts
from concourse.bass_utils import run_bass_kernel_spmd

NCORES = 8
N = 65536
VDIM = 2048
HDIM = 64
LN_EPS = 1e-5

NPC = N // NCORES          # rows per core = 8192
P = 128                    # partitions
TPC = NPC // P             # row-tiles per core = 64
GRP = 8                    # row-tiles per group (8*64 = 512 psum floats = 1 bank)
NG = TPC // GRP            # groups per core = 8
KC = VDIM // P             # contraction chunks = 16
RG = 512                   # rows per matmul moving operand (psum bank)

F32 = mybir.dt.float32
F16 = mybir.dt.float16


def _build_nc(has_b, has_g, has_be, tpc=TPC, ng=NG):
    """Build + compile the per-core Bass program (same NEFF on all cores)."""
    TPC, NG = tpc, ng  # noqa: N806 — allow small-scale builds for simulation
    nc = bacc.Bacc(
        "TRN2", target_bir_lowering=False, debug=False, num_devices=NCORES
    )

    NRG = TPC // 4  # 512-row halves per core
    vt = nc.dram_tensor("vt", [NRG, P, KC * RG], F16, kind="ExternalInput")
    w1 = nc.dram_tensor("w1", [VDIM, HDIM], F16, kind="ExternalInput")
    w2 = nc.dram_tensor("w2", [HDIM, HDIM], F16, kind="ExternalInput")
    w3 = nc.dram_tensor("w3", [HDIM, HDIM], F16, kind="ExternalInput")
    wg = nc.dram_tensor("wg", [NG, P, GRP, HDIM], F16, kind="ExternalInput")
    bg = nc.dram_tensor("bg", [NG, P, GRP], F32, kind="ExternalInput")
    ident = nc.dram_tensor("ident", [P, P], F16, kind="ExternalInput")
    b_in = g_in = be_in = None
    if has_b:
        b_in = nc.dram_tensor("bvec", [3, P, HDIM], F32, kind="ExternalInput")
    if has_g:
        g_in = nc.dram_tensor("gvec", [3, P, HDIM], F32, kind="ExternalInput")
    if has_be:
        be_in = nc.dram_tensor("bevec", [3, P, HDIM], F32, kind="ExternalInput")
    o = nc.dram_tensor("o", [NG, P, GRP], F32, kind="ExternalOutput")

    with tile.TileContext(nc) as tc:
        with (
            tc.tile_pool(name="consts", bufs=1) as consts,
            tc.tile_pool(name="vpool", bufs=8) as vpool,
            tc.tile_pool(name="xpool", bufs=8) as xpool,
            tc.tile_pool(name="upool", bufs=4) as upool,
            tc.tile_pool(name="sqpool", bufs=4) as sqpool,
            tc.tile_pool(name="xtpool", bufs=6) as xtpool,
            tc.tile_pool(name="wgpool", bufs=3) as wgpool,
            tc.tile_pool(name="bgpool", bufs=3) as bgpool,
            tc.tile_pool(name="stat", bufs=8) as stat,
            tc.tile_pool(name="respool", bufs=3) as respool,
            tc.tile_pool(name="pzt", bufs=3, space="PSUM") as pzt,
            tc.tile_pool(name="ppt", bufs=3, space="PSUM") as ppt,
            tc.tile_pool(name="psz", bufs=2, space="PSUM") as psz,
        ):
            # --- constants ---
            w1_sb = consts.tile([P, KC, HDIM], F16)
            nc.sync.dma_start(
                out=w1_sb[:], in_=w1[:].rearrange("(k p) h -> p k h", p=P)
            )
            w2_sb = consts.tile([HDIM, HDIM], F16)
            nc.sync.dma_start(out=w2_sb[:], in_=w2[:])
            w3_sb = consts.tile([HDIM, HDIM], F16)
            nc.sync.dma_start(out=w3_sb[:], in_=w3[:])
            id_sb = consts.tile([P, P], F16)
            nc.sync.dma_start(out=id_sb[:], in_=ident[:])
            eps_sb = consts.tile([P, 1], F32)
            nc.vector.memset(eps_sb[:], LN_EPS)
            b_sb = g_sb = be_sb = None
            if b_in is not None:
                b_sb = consts.tile([P, 3, HDIM], F32)
                nc.sync.dma_start(
                    out=b_sb[:], in_=b_in[:].rearrange("l p h -> p l h")
                )
            if g_in is not None:
                g_sb = consts.tile([P, 3, HDIM], F32)
                nc.sync.dma_start(
                    out=g_sb[:], in_=g_in[:].rearrange("l p h -> p l h")
                )
            if be_in is not None:
                be_sb = consts.tile([P, 3, HDIM], F32)
                nc.sync.dma_start(
                    out=be_sb[:], in_=be_in[:].rearrange("l p h -> p l h")
                )

            def ln_relu(pz, li):
                """LN (mean pre-folded into W) + relu: PSUM [P,GRP,H] -> SBUF."""
                w = pz
                if b_sb is not None:
                    wsb = upool.tile([P, GRP, HDIM], F32, tag="wsb")
                    nc.vector.tensor_add(
                        wsb[:],
                        pz[:],
                        b_sb[:, li, None, :].to_broadcast((P, GRP, HDIM)),
                    )
                    w = wsb
                sq = sqpool.tile([P, GRP, HDIM], F32)
                nc.scalar.square(sq[:], w[:])
                var = stat.tile([P, GRP], F32)
                nc.vector.reduce_sum(var[:], sq[:], axis=mybir.AxisListType.X)
                # std = sqrt(var/H + eps); inv = 1/std
                inv = stat.tile([P, GRP], F32)
                nc.scalar.activation(
                    inv[:],
                    var[:],
                    mybir.ActivationFunctionType.Sqrt,
                    bias=eps_sb[:],
                    scale=1.0 / HDIM,
                )
                nc.vector.reciprocal(inv[:], inv[:])
                u = upool.tile([P, GRP, HDIM], F32)
                nc.vector.tensor_mul(
                    u[:], w[:], inv[:, :, None].to_broadcast((P, GRP, HDIM))
                )
                if g_sb is not None:
                    nc.vector.tensor_mul(
                        u[:],
                        u[:],
                        g_sb[:, li, None, :].to_broadcast((P, GRP, HDIM)),
                    )
                if be_sb is not None:
                    nc.vector.tensor_add(
                        u[:],
                        u[:],
                        be_sb[:, li, None, :].to_broadcast((P, GRP, HDIM)),
                    )
                x = xpool.tile([P, GRP, HDIM], F16)
                nc.vector.tensor_scalar_max(x[:], u[:], 0.0)
                return x

            def pe_transpose(x):
                """x [P,GRP,H] -> pt [H,GRP,P] in PSUM via PE transposes."""
                pt = ppt.tile([HDIM, GRP, P], F16, tag="ppt")
                for t in range(GRP):
                    nc.tensor.transpose(pt[:, t, :], x[:, t, :], id_sb[:])
                return pt

            def mm23(xt, w_sb):
                """z = x @ W from transposed x; rows back on partitions."""
                pz2 = psz.tile([P, GRP, HDIM], F32, tag="pz")
                for t in range(GRP):
                    nc.tensor.matmul(
                        pz2[:, t, :],
                        lhsT=xt[:, t, :],
                        rhs=w_sb[:],
                        start=True,
                        stop=True,
                    )
                return pz2

            # ---- 4-stage modulo-scheduled pipeline over groups ----
            # Per-iteration emission order is hand-scheduled per engine:
            # PE [T3, T2, T1, MM3, MM2, L1] with the PSUM->SBUF copies
            # emitted right after their producing transposes, so the PE's
            # in-order queue always has cover work while a copy drains, and
            # long-waiting ops (LN1's square) sit at each queue's tail.
            st = {}
            for i in range(NG + 3):
                g3, g2, g1, g0 = i - 3, i - 2, i - 1, i
                s3 = st.get(g3) if 0 <= g3 < NG else None
                s2 = st.get(g2) if 0 <= g2 < NG else None
                s1 = st.get(g1) if 0 <= g1 < NG else None
                if g0 < NG:
                    # V loads first so the SP queue issues them ASAP
                    vhs = []
                    for half in range(2):
                        vh = vpool.tile([P, KC, RG], F16, tag="v")
                        nc.sync.dma_start(out=vh[:], in_=vt[2 * g0 + half])
                        vhs.append(vh)
                    st[g0] = {"vhs": vhs}

                # PE: transposes for stages 3 and 2
                pt3 = pe_transpose(s3["x"]) if s3 else None
                pt2 = pe_transpose(s2["x"]) if s2 else None
                # scalar: PSUM->SBUF copies (z1T copies have no wait at all)
                if s3:
                    xt3 = xtpool.tile([HDIM, GRP, P], F16, tag="xt")
                    nc.scalar.copy(xt3[:], pt3[:])
                if s2:
                    xt2 = xtpool.tile([HDIM, GRP, P], F16, tag="xt")
                    nc.scalar.copy(xt2[:], pt2[:])
                if s1:
                    z1T = xtpool.tile([HDIM, 2, RG], F16, tag="xt")
                    nc.scalar.copy(z1T[:, 0, :], s1["pzh"][0][:])
                    nc.scalar.copy(z1T[:, 1, :], s1["pzh"][1][:])
                # PE: T1 then the layer-2/3 matmuls (their copies are draining)
                if s1:
                    pz = ppt.tile([P, GRP, HDIM], F16, tag="ppt")
                    for t in range(GRP):
                        nc.tensor.transpose(
                            pz[:, t, :],
                            z1T[:, t // 4, ts(t % 4, P)],
                            id_sb[:HDIM, :HDIM],
                        )
                pz3 = mm23(xt3, w3_sb) if s3 else None
                pz2 = mm23(xt2, w2_sb) if s2 else None

                # scalar+DVE: LN chains oldest-first, then the head
                if s3:
                    x3 = ln_relu(pz3, 2)
                    scr = sqpool.tile([P, GRP, HDIM], F32, tag="scr")
                    nc.vector.tensor_mul(scr[:], x3[:], s3["wg"][:])
                    dot = stat.tile([P, GRP], F32, tag="dot")
                    nc.vector.reduce_sum(
                        dot[:], scr[:], axis=mybir.AxisListType.X
                    )
                    res = respool.tile([P, GRP], F32, tag="res")
                    nc.vector.tensor_add(res[:], dot[:], s3["bg"][:])
                    nc.sync.dma_start(out=o[g3], in_=res[:])
                    st.pop(g3)
                if s2:
                    s2["x"] = ln_relu(pz2, 1)
                if s1:
                    s1["x"] = ln_relu(pz, 0)
                    # prefetch head operands (used at stage 3)
                    wg_sb = wgpool.tile([P, GRP, HDIM], F16)
                    nc.sync.dma_start(out=wg_sb[:], in_=wg[g1])
                    bg_sb = bgpool.tile([P, GRP], F32)
                    nc.sync.dma_start(out=bg_sb[:], in_=bg[g1])
                    s1["wg"], s1["bg"] = wg_sb, bg_sb

                if g0 < NG:
                    # PE last: layer-1 matmuls (z^T halves in PSUM)
                    s = st[g0]
                    pzh = []
                    for half in range(2):
                        ph = pzt.tile([HDIM, RG], F32, tag="pzt")
                        for k in range(KC):
                            nc.tensor.matmul(
                                ph[:],
                                lhsT=w1_sb[:, k, :],
                                rhs=s["vhs"][half][:, k, :],
                                start=(k == 0),
                                stop=(k == KC - 1),
                            )
                        pzh.append(ph)
                    s["pzh"] = pzh

    nc.compile()
    return nc


_NC_CACHE = {}
LAST_RESULTS = None


def _get_nc(has_b, has_g, has_be):
    key = (has_b, has_g, has_be)
    if key not in _NC_CACHE:
        _NC_CACHE[key] = _build_nc(has_b, has_g, has_be)
    return _NC_CACHE[key]


def _center(w):
    # w @ (I - 1/H): subtract row-means, in float64 for exactness
    w64 = np.asarray(w, np.float64)
    return (w64 - w64.mean(axis=-1, keepdims=True)).astype(np.float32)


def kernel(
    V, ilist, temp, W1, b1, g1, be1, W2, b2, g2, be2, W3, b3, g3, be3,
    Wp, bp, marginals,
):
    V = np.asarray(V, np.float32)
    ilist_np = np.asarray(ilist)
    W1c = _center(np.asarray(W1)).astype(np.float16)
    W2c = _center(np.asarray(W2)).astype(np.float16)
    W3c = _center(np.asarray(W3)).astype(np.float16)
    bs = [np.asarray(b, np.float64) for b in (b1, b2, b3)]
    bs = np.stack([(b - b.mean()).astype(np.float32) for b in bs])  # [3, H]
    gs = np.stack([np.asarray(g, np.float32) for g in (g1, g2, g3)])
    bes = np.stack([np.asarray(b, np.float32) for b in (be1, be2, be3)])

    has_b = bool(np.any(bs))
    has_g = not bool(np.all(gs == 1.0))
    has_be = bool(np.any(bes))
    nc = _get_nc(has_b, has_g, has_be)

    # pre-gathered per-row output head
    Wg = np.ascontiguousarray(Wp[ilist_np, :, 0]).astype(np.float16)  # [N, H]
    bgv = np.ascontiguousarray(bp[ilist_np, 0, 0]).astype(np.float32)  # [N]

    shared = {
        "w1": W1c,
        "w2": W2c,
        "w3": W3c,
        "ident": np.eye(P, dtype=np.float16),
    }
    if has_b:
        shared["bvec"] = np.ascontiguousarray(
            np.broadcast_to(bs[:, None, :], (3, P, HDIM))
        )
    if has_g:
        shared["gvec"] = np.ascontiguousarray(
            np.broadcast_to(gs[:, None, :], (3, P, HDIM))
        )
    if has_be:
        shared["bevec"] = np.ascontiguousarray(
            np.broadcast_to(bes[:, None, :], (3, P, HDIM))
        )

    V16 = V.astype(np.float16)
    in_maps = []
    for c in range(NCORES):
        sl = slice(c * NPC, (c + 1) * NPC)
        # packed V^T row-halves: vt[rg, p, k*512 + r] = V[c*NPC + rg*512 + r, k*128 + p]
        vc = np.ascontiguousarray(
            V16[sl].reshape(TPC // 4, 512, KC, P).transpose(0, 3, 2, 1)
        ).reshape(TPC // 4, P, KC * 512)
        wgc = np.ascontiguousarray(
            Wg[sl].reshape(NG, GRP, P, HDIM).transpose(0, 2, 1, 3)
        )
        bgc = np.ascontiguousarray(
            bgv[sl].reshape(NG, GRP, P).transpose(0, 2, 1)
        )
        in_maps.append({"vt": vc, "wg": wgc, "bg": bgc, **shared})

    kres = run_bass_kernel_spmd(nc, in_maps, core_ids=list(range(NCORES)))
    global LAST_RESULTS
    LAST_RESULTS = kres
    out = np.empty(N, np.float32)
    for c in range(NCORES):
        oc = kres.results[c]["o"]  # [NG, P, GRP]
        out[c * NPC : (c + 1) * NPC] = oc.transpose(0, 2, 1).reshape(NPC)

    # epilogue on host: zero-row marginals + temperature
    zero_rows = np.abs(V).sum(axis=1) == 0.0
    if zero_rows.any():
        out = np.where(
            zero_rows, np.asarray(marginals, np.float32)[ilist_np], out
        )
    t = np.float32(np.asarray(temp))
    if t != 1.0:
        out = (out / t).astype(np.float32)
    return out


# revision 17
# speedup vs baseline: 2.5893x; 1.0121x over previous
"""Trainium2 Bass kernel for nn_DeltaAI_34703335752317 (gnn_message_passing).

Computation (see reference):
    x = relu(LN(V @ W1 + b1))   # [N, H], LN over H with eps=1e-5
    x = relu(LN(x @ W2 + b2))
    x = relu(LN(x @ W3 + b3))
    out[n] = dot(x[n], Wp[ilist[n], :, 0]) + bp[ilist[n]]
    out = where(sum|V[n]| == 0, marginals[ilist[n]], out) / temp

Strategy: pure data parallel over N across 8 cores.  Host pre-transposes V
(per-core packed [T, 128, VDIM] tiles so the contraction dim lands on SBUF
partitions with fully contiguous DMAs), folds the LN mean-centering into the
weights (z - mean(z) == V @ (W @ C) + b @ C with C = I - 1/H), and
pre-gathers the per-row output head Wp[ilist]/bp[ilist].  The device kernel
streams V^T tiles at HBM rate and runs matmuls + LN + head on chip.

All streamed data is fp16: halves HBM traffic vs fp32 and runs the PE at
1 cycle/row instead of fp32's 4.  PSUM accumulation and LN statistics stay
fp32.  (fp8 V was measured at 2.3e-2 max rel err — over the 2e-2 gate.)

The per-group work is modulo-scheduled across 4 pipeline stages
(L1 | T1+LN1 | T2+MM2+LN2 | T3+MM3+LN3+head) with a hand-chosen per-engine
emission order (PE: T3,T2,T1,MM3,MM2,L1; copies right after their producing
transposes) so the in-order engine queues never head-of-line-block the work
the PE needs early.
"""

import numpy as np

import concourse.bacc as bacc
import concourse.bass as bass
import concourse.tile as tile
from concourse import mybir
from concourse.bass import ts
from concourse.bass_utils import run_bass_kernel_spmd

NCORES = 8
N = 65536
VDIM = 2048
HDIM = 64
LN_EPS = 1e-5

NPC = N // NCORES          # rows per core = 8192
P = 128                    # partitions
TPC = NPC // P             # row-tiles per core = 64
GRP = 8                    # row-tiles per group (8*64 = 512 psum floats = 1 bank)
NG = TPC // GRP            # groups per core = 8
KC = VDIM // P             # contraction chunks = 16
RG = 512                   # rows per matmul moving operand (psum bank)

F32 = mybir.dt.float32
F16 = mybir.dt.float16


def _build_nc(has_b, has_g, has_be, tpc=TPC, ng=NG):
    """Build + compile the per-core Bass program (same NEFF on all cores)."""
    TPC, NG = tpc, ng  # noqa: N806 — allow small-scale builds for simulation
    nc = bacc.Bacc(
        "TRN2", target_bir_lowering=False, debug=False, num_devices=NCORES
    )

    NRG = TPC // 4  # 512-row halves per core
    vt = nc.dram_tensor("vt", [NRG, P, KC * RG], F16, kind="ExternalInput")
    w1 = nc.dram_tensor("w1", [VDIM, HDIM], F16, kind="ExternalInput")
    w2 = nc.dram_tensor("w2", [HDIM, HDIM], F16, kind="ExternalInput")
    w3 = nc.dram_tensor("w3", [HDIM, HDIM], F16, kind="ExternalInput")
    wg = nc.dram_tensor("wg", [NG, P, GRP, HDIM], F16, kind="ExternalInput")
    bg = nc.dram_tensor("bg", [NG, P, GRP], F32, kind="ExternalInput")
    ident = nc.dram_tensor("ident", [P, P], F16, kind="ExternalInput")
    b_in = g_in = be_in = None
    if has_b:
        b_in = nc.dram_tensor("bvec", [3, P, HDIM], F32, kind="ExternalInput")
    if has_g:
        g_in = nc.dram_tensor("gvec", [3, P, HDIM], F32, kind="ExternalInput")
    if has_be:
        be_in = nc.dram_tensor("bevec", [3, P, HDIM], F32, kind="ExternalInput")
    o = nc.dram_tensor("o", [NG, P, GRP], F32, kind="ExternalOutput")

    with tile.TileContext(nc) as tc:
        with (
            tc.tile_pool(name="consts", bufs=1) as consts,
            tc.tile_pool(name="vpool", bufs=8) as vpool,
            tc.tile_pool(name="xpool", bufs=8) as xpool,
            tc.tile_pool(name="upool", bufs=4) as upool,
            tc.tile_pool(name="sqpool", bufs=4) as sqpool,
            tc.tile_pool(name="xtpool", bufs=6) as xtpool,
            tc.tile_pool(name="wgpool", bufs=3) as wgpool,
            tc.tile_pool(name="bgpool", bufs=3) as bgpool,
            tc.tile_pool(name="stat", bufs=8) as stat,
            tc.tile_pool(name="respool", bufs=3) as respool,
            tc.tile_pool(name="pzt", bufs=3, space="PSUM") as pzt,
            tc.tile_pool(name="ppt", bufs=3, space="PSUM") as ppt,
            tc.tile_pool(name="psz", bufs=2, space="PSUM") as psz,
        ):
            # --- constants ---
            w1_sb = consts.tile([P, KC, HDIM], F16)
            nc.sync.dma_start(
                out=w1_sb[:], in_=w1[:].rearrange("(k p) h -> p k h", p=P)
            )
            w2_sb = consts.tile([HDIM, HDIM], F16)
            nc.sync.dma_start(out=w2_sb[:], in_=w2[:])
            w3_sb = consts.tile([HDIM, HDIM], F16)
            nc.sync.dma_start(out=w3_sb[:], in_=w3[:])
            id_sb = consts.tile([P, P], F16)
            nc.sync.dma_start(out=id_sb[:], in_=ident[:])
            eps_sb = consts.tile([P, 1], F32)
            nc.vector.memset(eps_sb[:], LN_EPS)
            b_sb = g_sb = be_sb = None
            if b_in is not None:
                b_sb = consts.tile([P, 3, HDIM], F32)
                nc.sync.dma_start(
                    out=b_sb[:], in_=b_in[:].rearrange("l p h -> p l h")
                )
            if g_in is not None:
                g_sb = consts.tile([P, 3, HDIM], F32)
                nc.sync.dma_start(
                    out=g_sb[:], in_=g_in[:].rearrange("l p h -> p l h")
                )
            if be_in is not None:
                be_sb = consts.tile([P, 3, HDIM], F32)
                nc.sync.dma_start(
                    out=be_sb[:], in_=be_in[:].rearrange("l p h -> p l h")
                )

            def ln_relu(pz, li):
                """LN (mean pre-folded into W) + relu: PSUM [P,GRP,H] -> SBUF."""
                w = pz
                if b_sb is not None:
                    wsb = upool.tile([P, GRP, HDIM], F32, tag="wsb")
                    nc.vector.tensor_add(
                        wsb[:],
                        pz[:],
                        b_sb[:, li, None, :].to_broadcast((P, GRP, HDIM)),
                    )
                    w = wsb
                sq = sqpool.tile([P, GRP, HDIM], F32)
                nc.scalar.square(sq[:], w[:])
                var = stat.tile([P, GRP], F32)
                nc.vector.reduce_sum(var[:], sq[:], axis=mybir.AxisListType.X)
                # std = sqrt(var/H + eps); inv = 1/std
                inv = stat.tile([P, GRP], F32)
                nc.scalar.activation(
                    inv[:],
                    var[:],
                    mybir.ActivationFunctionType.Sqrt,
                    bias=eps_sb[:],
                    scale=1.0 / HDIM,
                )
                nc.vector.reciprocal(inv[:], inv[:])
                u = upool.tile([P, GRP, HDIM], F32)
                nc.vector.tensor_mul(
                    u[:], w[:], inv[:, :, None].to_broadcast((P, GRP, HDIM))
                )
                if g_sb is not None:
                    nc.vector.tensor_mul(
                        u[:],
                        u[:],
                        g_sb[:, li, None, :].to_broadcast((P, GRP, HDIM)),
                    )
                if be_sb is not None:
                    nc.vector.tensor_add(
                        u[:],
                        u[:],
                        be_sb[:, li, None, :].to_broadcast((P, GRP, HDIM)),
                    )
                x = xpool.tile([P, GRP, HDIM], F16)
                nc.vector.tensor_scalar_max(x[:], u[:], 0.0)
                return x

            def pe_transpose(x):
                """x [P,GRP,H] -> pt [H,GRP,P] in PSUM via PE transposes."""
                pt = ppt.tile([HDIM, GRP, P], F16, tag="ppt")
                for t in range(GRP):
                    nc.tensor.transpose(pt[:, t, :], x[:, t, :], id_sb[:])
                return pt

            def mm23(xt, w_sb):
                """z = x @ W from transposed x; rows back on partitions."""
                pz2 = psz.tile([P, GRP, HDIM], F32, tag="pz")
                for t in range(GRP):
                    nc.tensor.matmul(
                        pz2[:, t, :],
                        lhsT=xt[:, t, :],
                        rhs=w_sb[:],
                        start=True,
                        stop=True,
                    )
                return pz2

            # ---- 4-stage modulo-scheduled pipeline over groups ----
            # Per-iteration emission order is hand-scheduled per engine:
            # PE [T3, T2, T1, MM3, MM2, L1] with the PSUM->SBUF copies
            # emitted right after their producing transposes, so the PE's
            # in-order queue always has cover work while a copy drains, and
            # long-waiting ops (LN1's square) sit at each queue's tail.
            st = {}
            for i in range(NG + 3):
                g3, g2, g1, g0 = i - 3, i - 2, i - 1, i
                s3 = st.get(g3) if 0 <= g3 < NG else None
                s2 = st.get(g2) if 0 <= g2 < NG else None
                s1 = st.get(g1) if 0 <= g1 < NG else None
                if g0 < NG:
                    # V loads first so the SP queue issues them ASAP
                    vhs = []
                    for half in range(2):
                        vh = vpool.tile([P, KC, RG], F16, tag="v")
                        nc.sync.dma_start(out=vh[:], in_=vt[2 * g0 + half])
                        vhs.append(vh)
                    st[g0] = {"vhs": vhs}

                # PE: transposes for stages 3 and 2
                pt3 = pe_transpose(s3["x"]) if s3 else None
                pt2 = pe_transpose(s2["x"]) if s2 else None
                # scalar: PSUM->SBUF copies (z1T copies have no wait at all)
                if s3:
                    xt3 = xtpool.tile([HDIM, GRP, P], F16, tag="xt")
                    nc.scalar.copy(xt3[:], pt3[:])
                if s2:
                    xt2 = xtpool.tile([HDIM, GRP, P], F16, tag="xt")
                    nc.scalar.copy(xt2[:], pt2[:])
                if s1:
                    z1T = xtpool.tile([HDIM, 2, RG], F16, tag="xt")
                    nc.scalar.copy(z1T[:, 0, :], s1["pzh"][0][:])
                    nc.scalar.copy(z1T[:, 1, :], s1["pzh"][1][:])
                # PE: T1 then the layer-2/3 matmuls (their copies are draining)
                if s1:
                    pz = ppt.tile([P, GRP, HDIM], F16, tag="ppt")
                    for t in range(GRP):
                        nc.tensor.transpose(
                            pz[:, t, :],
                            z1T[:, t // 4, ts(t % 4, P)],
                            id_sb[:HDIM, :HDIM],
                        )
                pz3 = mm23(xt3, w3_sb) if s3 else None
                pz2 = mm23(xt2, w2_sb) if s2 else None

                # scalar+DVE: LN chains oldest-first, then the head
                if s3:
                    x3 = ln_relu(pz3, 2)
                    scr = sqpool.tile([P, GRP, HDIM], F32, tag="scr")
                    nc.vector.tensor_mul(scr[:], x3[:], s3["wg"][:])
                    dot = stat.tile([P, GRP], F32, tag="dot")
                    nc.vector.reduce_sum(
                        dot[:], scr[:], axis=mybir.AxisListType.X
                    )
                    res = respool.tile([P, GRP], F32, tag="res")
                    nc.vector.tensor_add(res[:], dot[:], s3["bg"][:])
                    nc.sync.dma_start(out=o[g3], in_=res[:])
                    st.pop(g3)
                if s2:
                    s2["x"] = ln_relu(pz2, 1)
                if s1:
                    s1["x"] = ln_relu(pz, 0)
                    # prefetch head operands (used at stage 3)
                    wg_sb = wgpool.tile([P, GRP, HDIM], F16)
                    nc.sync.dma_start(out=wg_sb[:], in_=wg[g1])
                    bg_sb = bgpool.tile([P, GRP], F32)
                    nc.sync.dma_start(out=bg_sb[:], in_=bg[g1])
                    s1["wg"], s1["bg"] = wg_sb, bg_sb

                if g0 < NG:
                    # PE last: layer-1 matmuls (z^T halves in PSUM)
                    s = st[g0]
                    pzh = []
                    for half in range(2):
                        ph = pzt.tile([HDIM, RG], F32, tag="pzt")
                        for k in range(KC):
                            nc.tensor.matmul(
                                ph[:],
                                lhsT=w1_sb[:, k, :],
                                rhs=s["vhs"][half][:, k, :],
                                start=(k == 0),
                                stop=(k == KC - 1),
                            )
                        pzh.append(ph)
                    s["pzh"] = pzh

    nc.compile()
    return nc


_NC_CACHE = {}
LAST_RESULTS = None


def _get_nc(has_b, has_g, has_be):
    key = (has_b, has_g, has_be)
    if key not in _NC_CACHE:
        _NC_CACHE[key] = _build_nc(has_b, has_g, has_be)
    return _NC_CACHE[key]


def _center(w):
    # w @ (I - 1/H): subtract row-means, in float64 for exactness
    w64 = np.asarray(w, np.float64)
    return (w64 - w64.mean(axis=-1, keepdims=True)).astype(np.float32)


def kernel(
    V, ilist, temp, W1, b1, g1, be1, W2, b2, g2, be2, W3, b3, g3, be3,
    Wp, bp, marginals,
):
    V = np.asarray(V, np.float32)
    ilist_np = np.asarray(ilist)
    W1c = _center(np.asarray(W1)).astype(np.float16)
    W2c = _center(np.asarray(W2)).astype(np.float16)
    W3c = _center(np.asarray(W3)).astype(np.float16)
    bs = [np.asarray(b, np.float64) for b in (b1, b2, b3)]
    bs = np.stack([(b - b.mean()).astype(np.float32) for b in bs])  # [3, H]
    gs = np.stack([np.asarray(g, np.float32) for g in (g1, g2, g3)])
    bes = np.stack([np.asarray(b, np.float32) for b in (be1, be2, be3)])

    has_b = bool(np.any(bs))
    has_g = not bool(np.all(gs == 1.0))
    has_be = bool(np.any(bes))
    nc = _get_nc(has_b, has_g, has_be)

    # pre-gathered per-row output head
    Wg = np.ascontiguousarray(Wp[ilist_np, :, 0]).astype(np.float16)  # [N, H]
    bgv = np.ascontiguousarray(bp[ilist_np, 0, 0]).astype(np.float32)  # [N]

    shared = {
        "w1": W1c,
        "w2": W2c,
        "w3": W3c,
        "ident": np.eye(P, dtype=np.float16),
    }
    if has_b:
        shared["bvec"] = np.ascontiguousarray(
            np.broadcast_to(bs[:, None, :], (3, P, HDIM))
        )
    if has_g:
        shared["gvec"] = np.ascontiguousarray(
            np.broadcast_to(gs[:, None, :], (3, P, HDIM))
        )
    if has_be:
        shared["bevec"] = np.ascontiguousarray(
            np.broadcast_to(bes[:, None, :], (3, P, HDIM))
        )

    V16 = V.astype(np.float16)
    in_maps = []
    for c in range(NCORES):
        sl = slice(c * NPC, (c + 1) * NPC)
        # packed V^T row-halves: vt[rg, p, k*512 + r] = V[c*NPC + rg*512 + r, k*128 + p]
        vc = np.ascontiguousarray(
            V16[sl].reshape(TPC // 4, 512, KC, P).transpose(0, 3, 2, 1)
        ).reshape(TPC // 4, P, KC * 512)
        wgc = np.ascontiguousarray(
            Wg[sl].reshape(NG, GRP, P, HDIM).transpose(0, 2, 1, 3)
        )
        bgc = np.ascontiguousarray(
            bgv[sl].reshape(NG, GRP, P).transpose(0, 2, 1)
        )
        in_maps.append({"vt": vc, "wg": wgc, "bg": bgc, **shared})

    kres = run_bass_kernel_spmd(nc, in_maps, core_ids=list(range(NCORES)))
    global LAST_RESULTS
    LAST_RESULTS = kres
    out = np.empty(N, np.float32)
    for c in range(NCORES):
        oc = kres.results[c]["o"]  # [NG, P, GRP]
        out[c * NPC : (c + 1) * NPC] = oc.transpose(0, 2, 1).reshape(NPC)

    # epilogue on host: zero-row marginals + temperature
    zero_rows = np.abs(V).sum(axis=1) == 0.0
    if zero_rows.any():
        out = np.where(
            zero_rows, np.asarray(marginals, np.float32)[ilist_np], out
        )
    t = np.float32(np.asarray(temp))
    if t != 1.0:
        out = (out / t).astype(np.float32)
    return out
